# revision 25
# baseline (speedup 1.0000x reference)
"""Expert-parallel MoE FFN kernel for Trainium2 (8 NeuronCores).

Reference computation (per expert e):
    y[:, e*C:(e+1)*C, :] = gelu(x_e @ w1[e] + b1[e]) @ w2[e] + b2[e]

Sharding: expert-parallel - core e owns expert e (E == n_cores == 8) and the
matching chunk of dim 1 of `inputs`. No cross-core communication.

Per-core dataflow (T=16384 tokens, D=512, F=2048), all matmuls bf16:
  - X is pre-transposed AND pre-cast to bf16 on the host: the device
    receives X^T laid out [128d, DSUB, T] so mm1's moving operand streams
    straight from HBM with a single contiguous-ish DMA per 512-token chunk
    (no on-device cast, no DRAM bounce, no XBAR transpose). This removes
    the 35us pipeline-fill delay and the early-chunk starvation stalls the
    bounce-transpose chain caused (each stall also re-throttled the PE's
    HAM clock gate to 1.2 GHz for ~3.4us windows).
  - A short burst of dummy matmuls on a memset tile warms the HAM clock
    gate (K=4/8 -> 8/8) while the first weight/x DMAs are in flight, so
    real matmuls start at full 2.4 GHz.
  - mm1: H^T[f, t] += W1[d, f].T @ X^T[d, t]; gelu+b1 fused on ScalarE
    (f on partitions -> b1 is a per-partition bias), H stored bf16.
  - mm2: Y[t, d] += (H^T[f, t128]).T @ W2[f, d] with H^T as the stationary
    operand, so Y comes out token-major and stores contiguously.

PE floor for this shape at warm clock: 4096 MMs x ~216 ns = ~884 us/core.
"""

import numpy as np
import ml_dtypes

import concourse.bacc as bacc
import concourse.bass as bass
import concourse.mybir as mybir
import concourse.tile as tile
from concourse.bass_utils import run_bass_kernel_spmd

B, EC, D = 16, 8192, 512
E, F = 8, 2048
C = EC // E            # capacity per expert = 1024
T = B * C              # tokens per expert/core = 16384
P = 128
DSUB = D // P          # 4
FSUB = F // P          # 16
TCHUNK = 512
TS = TCHUNK // P       # 4
N_CORES = 8
NDUMMY = 20            # HAM warm-up matmuls (bridge until first x/w DMAs land)

# Stash of the last BassKernelResults (for test harness profiling).
LAST_RESULT = None


def build_nc(n_tokens: int = T, act_func=None):
    if act_func is None:
        act_func = mybir.ActivationFunctionType.Gelu_apprx_tanh
    nchunk = n_tokens // TCHUNK
    nc = bacc.Bacc(
        "TRN2",
        target_bir_lowering=False,
        debug=False,
        num_devices=N_CORES,
    )
    xt = nc.dram_tensor("xt", [P, nchunk, DSUB, TCHUNK], mybir.dt.bfloat16, kind="ExternalInput").ap()
    w1 = nc.dram_tensor("w1", [P, FSUB, DSUB, P], mybir.dt.bfloat16, kind="ExternalInput").ap()
    b1 = nc.dram_tensor("b1", [P, FSUB], mybir.dt.float32, kind="ExternalInput").ap()
    w2 = nc.dram_tensor("w2", [P, FSUB, D], mybir.dt.bfloat16, kind="ExternalInput").ap()
    b2 = nc.dram_tensor("b2", [P, D], mybir.dt.float32, kind="ExternalInput").ap()
    y = nc.dram_tensor("y", [n_tokens, D], mybir.dt.float32, kind="ExternalOutput").ap()

    with tile.TileContext(nc) as tc:
        with (
            tc.tile_pool(name="consts", bufs=1) as consts,
            tc.tile_pool(name="xt", bufs=6) as xt_pool,
            tc.tile_pool(name="h", bufs=3) as h_pool,
            tc.tile_pool(name="yout", bufs=4) as y_pool,
            tc.tile_pool(name="ps_h", bufs=3, space="PSUM") as ps_h,
            tc.tile_pool(name="ps_y", bufs=3, space="PSUM") as ps_y,
            tc.tile_pool(name="ps_warm", bufs=1, space="PSUM") as ps_warm,
        ):
            # HAM clock warm-up: matmuls on a zeroed tile keep the PE busy
            # from ~t=2us while the first DMAs land, so the activity monitor
            # releases the clock gate before real work begins.
            wrm = consts.tile([P, TCHUNK], mybir.dt.bfloat16)
            nc.vector.memset(wrm, 0.0)
            ps_w = ps_warm.tile([P, TCHUNK], mybir.dt.float32)
            for _ in range(NDUMMY):
                nc.tensor.matmul(ps_w, lhsT=wrm[:, :P], rhs=wrm, start=True, stop=True)
            # Touch the activation function once so its ACT table loads now,
            # not lazily inside the first real gelu (which stalled mm1 once).
            act_wrm = consts.tile([P, 8], mybir.dt.float32)
            nc.scalar.activation(act_wrm, wrm[:, :8], act_func)

            # Startup DMA plan: the two HWDGE queues each run ~130-150 GB/s,
            # so balance the ~4.8MB of startup traffic across both in the
            # order the PE consumes it:
            #   sync:   xt0[ds0], xt0[ds2], w1 g0..g3, xt chunks 1..31
            #   scalar: b1, b2, xt0[ds1], xt0[ds3], w2 g0..g3, y stores
            b1_sb = consts.tile([P, FSUB], mybir.dt.float32)
            nc.scalar.dma_start(b1_sb, b1)
            b2_sb = consts.tile([P, D], mybir.dt.float32)
            nc.scalar.dma_start(b2_sb, b2)

            xt0_sb = xt_pool.tile([P, DSUB, TCHUNK], mybir.dt.bfloat16)
            for ds, eng in enumerate((nc.sync, nc.scalar, nc.sync, nc.scalar)):
                eng.dma_start(xt0_sb[:, ds, :], xt[:, 0, ds, :])

            # w1 laid out [P, FSUB, DSUB, 128]: each f-tile group is one
            # contiguous 4KB-per-partition read (big DMA packets), arriving
            # in the order mm1 consumes f-tiles.
            w1_sb = consts.tile([P, FSUB, DSUB, P], mybir.dt.bfloat16)
            for lo, hi in ((0, 2), (2, 4), (4, 8), (8, 12), (12, 16)):
                nc.sync.dma_start(w1_sb[:, lo:hi], w1[:, lo:hi])
            w2_sb = consts.tile([P, FSUB, D], mybir.dt.bfloat16)
            for i in range(4):
                nc.scalar.dma_start(
                    w2_sb[:, i * 4:(i + 1) * 4, :], w2[:, i * 4:(i + 1) * 4, :]
                )

            for c in range(nchunk):
                # One 512-token chunk of X^T, already bf16: single DMA
                # (chunk 0 was prefetched above, split across two queues).
                if c == 0:
                    xt_sb = xt0_sb
                else:
                    xt_sb = xt_pool.tile([P, DSUB, TCHUNK], mybir.dt.bfloat16)
                    nc.sync.dma_start(xt_sb, xt[:, c])

                # mm1 + fused gelu/bias: H^T[f, t] bf16.
                h = h_pool.tile([P, FSUB, TCHUNK], mybir.dt.bfloat16)
                for fs in range(FSUB):
                    ph = ps_h.tile([P, TCHUNK], mybir.dt.float32)
                    for ds in range(DSUB):
                        nc.tensor.matmul(
                            ph,
                            lhsT=w1_sb[:, fs, ds],
                            rhs=xt_sb[:, ds, :],
                            start=(ds == 0),
                            stop=(ds == DSUB - 1),
                        )
                    nc.scalar.activation(
                        h[:, fs, :],
                        ph,
                        act_func,
                        bias=b1_sb[:, fs:fs + 1],
                        scale=1.0,
                    )

                # mm2: Y[t, d] per 128-token subtile; + b2; store.
                for ts in range(TS):
                    py = ps_y.tile([P, D], mybir.dt.float32)
                    y_sb = y_pool.tile([P, D], mybir.dt.float32)
                    r0 = c * TCHUNK + ts * P
                    if c == nchunk - 1 and ts == TS - 1:
                        # Very last output tile: compute/flush it in column
                        # splits (384 + 128) on separate queues so only a
                        # 64KB store chain trails the final matmul.
                        for lo, hi, eng in ((0, 384, nc.scalar), (384, D, nc.sync)):
                            for fs in range(FSUB):
                                nc.tensor.matmul(
                                    py[:, lo:hi],
                                    lhsT=h[:, fs, ts * P:(ts + 1) * P],
                                    rhs=w2_sb[:, fs, lo:hi],
                                    start=(fs == 0),
                                    stop=(fs == FSUB - 1),
                                )
                            nc.vector.tensor_add(
                                y_sb[:, lo:hi], py[:, lo:hi], b2_sb[:, lo:hi]
                            )
                            eng.dma_start(y[r0:r0 + P, lo:hi], y_sb[:, lo:hi])
                    else:
                        for fs in range(FSUB):
                            nc.tensor.matmul(
                                py,
                                lhsT=h[:, fs, ts * P:(ts + 1) * P],
                                rhs=w2_sb[:, fs, :],
                                start=(fs == 0),
                                stop=(fs == FSUB - 1),
                            )
                        nc.vector.tensor_add(y_sb, py, b2_sb)
                        nc.scalar.dma_start(y[r0:r0 + P, :], y_sb)

    nc.compile()
    return nc


_NC_CACHE = {}


def _get_nc(n_tokens: int = T):
    if n_tokens not in _NC_CACHE:
        _NC_CACHE[n_tokens] = build_nc(n_tokens)
    return _NC_CACHE[n_tokens]


def make_in_maps(inputs, w1, b1, w2, b2):
    """Shard + lay out host-side: core e gets expert e."""
    bf16 = ml_dtypes.bfloat16
    inputs = np.asarray(inputs)
    w1, b1 = np.asarray(w1), np.asarray(b1)
    w2, b2 = np.asarray(w2), np.asarray(b2)
    in_maps = []
    for e in range(E):
        # x_e [T, D] -> X^T as [P, NCHUNK, DSUB, TCHUNK] bf16 with
        # d = ds*128 + p, t = c*TCHUNK + t' (4KB contiguous per partition
        # per chunk -> large DMA packets).
        x_e = inputs[:, e * C:(e + 1) * C, :].reshape(T, D).astype(bf16)
        xt_e = np.ascontiguousarray(
            x_e.reshape(T // TCHUNK, TCHUNK, DSUB, P).transpose(3, 0, 2, 1)
        )
        # w1[e] [D, F] -> [P, FSUB, DSUB, 128] with d = ds*128 + p,
        # f = fs*128 + fi (per-f-tile contiguous per partition).
        w1_e = np.ascontiguousarray(
            w1[e].reshape(DSUB, P, FSUB, P).transpose(1, 2, 0, 3).astype(bf16)
        )
        # b1[e] [F] -> [P, FSUB] with f = fs*128 + p
        b1_e = np.ascontiguousarray(
            b1[e].reshape(FSUB, P).T.astype(np.float32)
        )
        # w2[e] [F, D] -> [P, FSUB, D] with f = fs*128 + p
        w2_e = np.ascontiguousarray(
            w2[e].reshape(FSUB, P, D).transpose(1, 0, 2).astype(bf16)
        )
        # b2[e] [D] -> broadcast to [P, D]
        b2_e = np.ascontiguousarray(
            np.broadcast_to(b2[e].astype(np.float32), (P, D))
        )
        in_maps.append(
            {"xt": xt_e, "w1": w1_e, "b1": b1_e, "w2": w2_e, "b2": b2_e}
        )
    return in_maps


def kernel(inputs, w1, b1, w2, b2):
    global LAST_RESULT
    nc = _get_nc(T)
    in_maps = make_in_maps(inputs, w1, b1, w2, b2)
    res = run_bass_kernel_spmd(nc, in_maps, core_ids=list(range(N_CORES)))
    LAST_RESULT = res
    out = np.empty((B, EC, D), dtype=np.float32)
    for e in range(E):
        out[:, e * C:(e + 1) * C, :] = res.results[e]["y"].reshape(B, C, D)
    return out
